# revision 48
# baseline (speedup 1.0000x reference)
"""Fused multi-head attention with LSE for Trainium2 (Bass/Tile).

Problem: B=1, H=16, S=4096, D=64 fp32; additive HF-style mask [1,1,1,S].
Returns (context [1,16,4096,64], lse [1,16,4096,1]).

Sharding: tensor-parallel over heads — 2 heads per NeuronCore on 8 cores.
Each core computes its heads' full SxS attention independently; the host
concatenates the per-core outputs along H.

Per-head algorithm on one core (fp32 storage, float32r matmuls):
  - Load Q, K in natural [S, D] layout, PE-transpose 128x64 blocks into
    QT/KT [D, S] (d on partitions) for the QK^T matmul.
  - Load V as [128, S/128, D+1] with a ones column appended (V').
  - For each q chunk (1024 wide) and k-block kb (128 rows):
      scores^T tile [k=128, q=1024] = matmul(lhsT=KT_kb, rhs=QT[:, qtile])
      E^T = exp(scores^T / sqrt(D) + mask[k])  (one ScalarE op; mask is the
            per-partition bias, the scale folds into the activation)
      ctx^T[0:D+1, q] += matmul(lhsT=V'_kb, rhs=E^T)   (PSUM accumulation)
  - ctx^T row D is sum_k exp(scores+mask) = exp(lse); rows 0..D-1 are the
    unnormalized context^T. Ln of that row gives lse directly; PE-transpose
    back to [q, d] and multiply by the reciprocal of row D for the context.

The softmax denominator comes from the matmul ones-column (no partition-axis
reduction anywhere), and no max-subtraction is needed: scores ~ N(0,1) after
scaling, so exp is far from fp32 overflow.

ScalarE (exp at 1 elem/lane/cycle @ 1.2 GHz) is the bottleneck engine
(~219us of pure exp per core), so the whole schedule is built to keep it
saturated:
  - matmuls run as float32r (1 cycle/row; real fp32 is 4) so the PE stays
    under the ScalarE time,
  - PV emission is delayed one tile so the in-order PE queue never stalls
    on the exp it feeds,
  - per-chunk epilogues and the next head's prologue are chopped into small
    closures injected one-per-kb into the steady-state loop (a chunk's
    transpose/normalize runs during the NEXT chunk's matmuls),
  - all Ln calls are pinned to the kernel tail (Ln lives in a different ACT
    table set than Exp; a set switch costs ~2.7us, so interleaving would
    thrash the table loads).

PSUM (8 banks): scores [128,1024] x2 bufs (4) + ctx^T [65,1024] (2) +
transpose staging (2).
"""

import math
import os

os.environ.setdefault("JAX_PLATFORMS", "axon")

import numpy as np

import concourse.bacc as bacc
import concourse.bass as bass
import concourse.mybir as mybir
import concourse.tile as tile
from concourse.bass_utils import run_bass_kernel_spmd


P = 128
D = 64
FULL_S = 4096
H = 16
N_CORES = 8
HPC = H // N_CORES  # heads per core

F32 = mybir.dt.float32
# float32r: same 4-byte layout, but the PE runs it single-pass (1 cycle/row
# vs full fp32's 4 — fp32 needs 2 half-speed passes). Slightly reduced
# multiply precision; fp32 PSUM accumulation.
F32R = mybir.dt.float32r
AF = mybir.ActivationFunctionType

_BASS_CACHE = {}

# Exp and Ln both live in the "natural_log_exp_and_others" ACT table set,
# but bacc's per-function chooser would pick "exp_and_others" for Exp and
# "natural_log" for Ln — alternating ~2.7us table loads. Editing the
# candidate sets (NOT their order — act_func_set_id is the dict index) so
# Exp/Ln only appear in the combined set forces one load for the whole
# kernel and lets Ln calls interleave freely with the exp stream. Applied
# only around this module's nc.compile() call.
class _single_act_table:
    def __enter__(self):
        self._orig = bacc.get_activation_tables

        def patched(arch):
            tables = self._orig(arch)
            both = "natural_log_exp_and_others"
            if both in tables and AF.Exp in tables[both] and AF.Ln in tables[both]:
                for name, funcs in tables.items():
                    if name != both:
                        funcs.discard(AF.Exp)
                        funcs.discard(AF.Ln)
            return tables

        bacc.get_activation_tables = patched

    def __exit__(self, *exc):
        bacc.get_activation_tables = self._orig


def build_attention_bass(S=FULL_S, qc=1024, e_bf16=True):
    """Build the per-core Bass program (SPMD across 8 cores).

    e_bf16: store exp results and V' in bf16. HW-measured: the ScalarE exp
    stream runs 1.22x faster with bf16 output (646us -> 528us per core), and
    bf16 PV matmuls keep the same PE rate. Cost: context rel err ~5e-3
    instead of ~4e-4 (still far inside tolerance for this problem).
    """
    key = (S, qc, e_bf16)
    if key in _BASS_CACHE:
        return _BASS_CACHE[key]
    edt = mybir.dt.bfloat16 if e_bf16 else F32R

    qc = min(qc, S)
    n_kb = S // P       # k blocks of 128
    n_qc = S // qc      # q chunks (ctx^T accumulator width)
    n_pv = qc // 512    # matmuls per score tile (PSUM bank = 512 fp32)
    n_tb = qc // P      # transpose blocks per chunk epilogue
    scale = 1.0 / math.sqrt(D)

    # Bacc (not raw Bass): its compile() runs generate_event_semaphores(),
    # which splits multi-wait instructions — TRN2 allows only ONE sync wait
    # per instruction (walrus dies with "too many sync wait commands" on the
    # raw-Bass path whenever Tile hangs two fresh waits on a matmul).
    nc = bacc.Bacc()
    q_d = nc.dram_tensor("q", [HPC, S, D], F32, kind="ExternalInput")
    k_d = nc.dram_tensor("k", [HPC, S, D], F32, kind="ExternalInput")
    v_d = nc.dram_tensor("v", [HPC, S, D], F32, kind="ExternalInput")
    m_d = nc.dram_tensor("mask", [S], F32, kind="ExternalInput")
    ctx_d = nc.dram_tensor("ctx", [HPC, S, D], F32, kind="ExternalOutput")
    lse_d = nc.dram_tensor("lse", [HPC, S], F32, kind="ExternalOutput")

    with tile.TileContext(nc) as tc:
        with (
            tc.tile_pool(name="const", bufs=1) as cpool,
            tc.tile_pool(name="head", bufs=2) as hpool,
            tc.tile_pool(name="work", bufs=2) as wpool,
            tc.tile_pool(name="psum", bufs=2, space="PSUM") as ppool,
        ):
            identity = cpool.tile([P, P], F32)
            nc.gpsimd.memset(identity, 0.0)
            nc.gpsimd.affine_select(
                out=identity,
                in_=identity,
                compare_op=mybir.AluOpType.not_equal,
                fill=1.0,
                base=0,
                pattern=[[-1, P]],
                channel_multiplier=1,
            )
            mask_sb = cpool.tile([P, n_kb], F32)
            with nc.allow_non_contiguous_dma(reason="one-time 16KB mask load"):
                nc.sync.dma_start(
                    out=mask_sb, in_=m_d[:].rearrange("(n p) -> p n", p=P)
                )

            # Deferred-work queue: closures injected one-per-kb into the
            # steady-state loop so PE-side epilogue/prologue work interleaves
            # with matmuls instead of stalling the in-order PE queue.
            inject = []

            def drain(n=1):
                for _ in range(n):
                    if inject:
                        inject.pop(0)()

            heads = {}

            def emit_prologue(h, first=False):
                # q_nat/k_nat share one tag: they are transient (released
                # after their transposes), and SBUF is tight.
                q_nat = hpool.tile([P, n_kb, D], F32, tag="nat", name=f"q_nat{h}")
                nc.sync.dma_start(
                    out=q_nat, in_=q_d[h].rearrange("(n p) d -> p n d", p=P)
                )
                k_nat = hpool.tile([P, n_kb, D], F32, tag="nat", name=f"k_nat{h}")
                nc.sync.dma_start(
                    out=k_nat, in_=k_d[h].rearrange("(n p) d -> p n d", p=P)
                )
                vv_raw = hpool.tile([P, n_kb, D + 1], F32, tag="nat", name=f"vr{h}")
                nc.sync.dma_start(
                    out=vv_raw[:, :, 0:D],
                    in_=v_d[h].rearrange("(n p) d -> p n d", p=P),
                )
                nc.vector.memset(vv_raw[:, :, D : D + 1], 1.0)
                # fp32r matmul operands must be pre-rounded by their producer
                # (walrus birverifier checkMatmultFP32r); for bf16 the copy
                # is the narrowing cast.
                vv = hpool.tile([P, n_kb, D + 1], edt, tag="vv", name=f"vv{h}")
                nc.vector.tensor_copy(out=vv, in_=vv_raw)
                # QK runs as TWO concurrent matmuls on the PE's row halves
                # (tile_position row packing — contraction is only 64, so a
                # single matmul wastes half the array). kp stacks k-block
                # pairs on partition halves; qd holds QT duplicated on both
                # halves so each half-matmul streams its own copy.
                qd = hpool.tile([P, n_kb, P], F32R, tag="qt", name=f"qd{h}")
                kp = hpool.tile([P, n_kb // 2, P], F32R, tag="kt", name=f"kp{h}")
                heads[h] = (qd, kp, vv)

                def q_group(g):
                    # Transpose outputs must land on PSUM partition 0
                    # (walrus NCC_IBIR151), so the partition-64..127 copy of
                    # QT is made with an SBUF->SBUF DMA (DMA can shift
                    # partitions; engines cannot).
                    ng = min(4, n_kb - g)

                    def emit():
                        tp = ppool.tile([D, 4, P], F32, tag="tp", name="tp")
                        for j in range(ng):
                            nc.tensor.transpose(
                                tp[:, j, :], q_nat[:, g + j, :], identity
                            )
                        nc.vector.tensor_copy(
                            out=qd[0:D, g : g + ng, :], in_=tp[:, 0:ng, :]
                        )
                        nc.sync.dma_start(
                            out=qd[D : 2 * D, g : g + ng, :],
                            in_=qd[0:D, g : g + ng, :],
                        )

                    return emit

                def k_group(b):
                    # blocks 4b..4b+3 -> pairs 2b, 2b+1: even blocks on
                    # partitions 0-63 (DVE copy), odd blocks bounced through
                    # SBUF and DMA-shifted to partitions 64-127.
                    def emit():
                        tp = ppool.tile([D, 4, P], F32, tag="tp", name="tp")
                        for j in range(4):
                            nc.tensor.transpose(
                                tp[:, j, :], k_nat[:, 4 * b + j, :], identity
                            )
                        tp_pairs = tp.rearrange("d (p two) x -> d two p x", two=2)
                        nc.vector.tensor_copy(
                            out=kp[0:D, 2 * b : 2 * b + 2, :], in_=tp_pairs[:, 0]
                        )
                        kbounce = hpool.tile(
                            [D, 2, P], F32R, tag="kbounce", name="kbounce"
                        )
                        nc.vector.tensor_copy(out=kbounce, in_=tp_pairs[:, 1])
                        nc.sync.dma_start(
                            out=kp[D : 2 * D, 2 * b : 2 * b + 2, :], in_=kbounce
                        )

                    return emit

                qgroups = [q_group(g) for g in range(0, n_kb, 4)]
                kgroups = [k_group(b) for b in range(n_kb // 4)]
                if first:
                    # Emit only what chunk 0 needs up front (qt covering its
                    # q window + the first k pairs) so the PE reaches the
                    # first QK matmul ~2us in, not after all transpose groups.
                    # The rest drains inside the loop, kt-first since k
                    # group b covers pairs consumed from pair 2b on.
                    nq_up = min(len(qgroups), (qc + 511) // 512)
                    for emit in qgroups[:nq_up] + kgroups[:1]:
                        emit()
                    inject.extend(kgroups[1:] + qgroups[nq_up:])
                else:
                    inject.extend(kgroups + qgroups)

            def emit_epilogue_tail(h, qci, ctx_sb):
                """Transpose-back + normalize + store, chopped into closures."""
                out_sb = wpool.tile(
                    [P, n_tb, D], F32, tag="out_sb", name=f"out{h}_{qci}"
                )

                def block(t_i):
                    def emit():
                        tps = ppool.tile([P, P], F32, tag="tp", name="tps")
                        nc.tensor.transpose(
                            tps[:, 0 : D + 1],
                            ctx_sb[:, t_i * P : (t_i + 1) * P],
                            identity[: D + 1, : D + 1],
                        )
                        recip = wpool.tile([P, 1], F32, tag="recip", bufs=4, name="recip")
                        nc.vector.reciprocal(recip, tps[:, D : D + 1])
                        nc.vector.tensor_scalar_mul(
                            out_sb[:, t_i, :], tps[:, 0:D], recip
                        )

                    return emit

                def store():
                    nc.sync.dma_start(
                        out=ctx_d[h, qci * qc : (qci + 1) * qc, :].rearrange(
                            "(n p) d -> p n d", p=P
                        ),
                        in_=out_sb,
                    )

                for t_i in range(n_tb):
                    inject.append(block(t_i))
                inject.append(store)

            pending_lse = []
            emit_prologue(0, first=True)
            for h in range(HPC):
                qd, kp, vv = heads[h]
                for qci in range(n_qc):
                    ctx_ps = ppool.tile(
                        [D + 1, qc], F32, tag="ctx", bufs=1, name="ctx_ps"
                    )
                    if h + 1 < HPC and qci == n_qc - 1:
                        emit_prologue(h + 1)  # DMAs now; transposes via inject
                    pending_pv = []
                    q0 = qci * qc
                    for p in range(n_kb // 2):
                        sA = ppool.tile([P, qc], F32, tag="s", name="sA")
                        sB = ppool.tile([P, qc], F32, tag="s", name="sB")
                        for j in range(n_pv):
                            win = slice(
                                (q0 + j * 512) // P, (q0 + (j + 1) * 512) // P
                            )
                            # Two half-array matmuls, concurrent on row
                            # groups 0-63 / 64-127 (contraction is 64).
                            nc.tensor.matmul(
                                sA[:, j * 512 : (j + 1) * 512],
                                lhsT=kp[0:D, p, :],
                                rhs=qd[0:D, win, :],
                                start=True,
                                stop=True,
                            )
                            nc.tensor.matmul(
                                sB[:, j * 512 : (j + 1) * 512],
                                lhsT=kp[D : 2 * D, p, :],
                                rhs=qd[D : 2 * D, win, :],
                                start=True,
                                stop=True,
                                tile_position=(D, 0),
                            )
                        eA = wpool.tile([P, qc], edt, tag="e", bufs=4, name="eA")
                        nc.scalar.activation(
                            eA, sA, AF.Exp,
                            bias=mask_sb[:, 2 * p : 2 * p + 1], scale=scale,
                        )
                        eB = wpool.tile([P, qc], edt, tag="e", bufs=4, name="eB")
                        nc.scalar.activation(
                            eB, sB, AF.Exp,
                            bias=mask_sb[:, 2 * p + 1 : 2 * p + 2], scale=scale,
                        )
                        for item in pending_pv:
                            _emit_pv(nc, ctx_ps, vv, item, n_kb, n_pv)
                        pending_pv = [(eA, 2 * p), (eB, 2 * p + 1)]
                        drain(2)
                    for item in pending_pv:
                        _emit_pv(nc, ctx_ps, vv, item, n_kb, n_pv)

                    # Free the ctx PSUM slot ASAP; the rest of the epilogue is
                    # injected into the next chunk's loop.
                    ctx_sb = wpool.tile(
                        [D + 1, qc], F32, tag="ctx_sb", bufs=2 * n_qc,
                        name=f"ctx_sb{h}_{qci}",
                    )
                    nc.vector.tensor_copy(out=ctx_sb, in_=ctx_ps)
                    emit_epilogue_tail(h, qci, ctx_sb)
                    pending_lse.append((h, qci, ctx_sb))

            drain(len(inject))  # flush the final chunk's epilogue

            # lse = ln(denominator row), in place on ctx_sb row D (ACT lanes
            # are partition-aligned). With the act-table patch above, Ln
            # shares the exp's table set, so the scheduler may slot these
            # into idle ACT moments at no extra cost.
            for h, qci, ctx_sb in pending_lse:
                nc.scalar.activation(
                    ctx_sb[D : D + 1, :], ctx_sb[D : D + 1, :], AF.Ln
                )
                nc.sync.dma_start(
                    out=lse_d[h, qci * qc : (qci + 1) * qc],
                    in_=ctx_sb[D : D + 1, :],
                )

    with _single_act_table():
        nc.compile()  # bacc legalization: wait split, nop fusion, act tables
    _BASS_CACHE[key] = nc
    return nc


def _emit_pv(nc, ctx_ps, vv, pending, n_kb, n_pv):
    e_sb, kb = pending
    for j in range(n_pv):
        off = j * 512
        nc.tensor.matmul(
            ctx_ps[:, off : off + 512],
            lhsT=vv[:, kb, :],
            rhs=e_sb[:, off : off + 512],
            start=(kb == 0),
            stop=(kb == n_kb - 1),
        )


def run_cores(q, k, v, mask, S=FULL_S, trace=False, **kw):
    """q/k/v: [H, S, D] fp32; mask: [S] fp32. Returns (results, BassKernelResults)."""
    nc = build_attention_bass(S=S)
    in_maps = []
    for c in range(N_CORES):
        hs = slice(c * HPC, (c + 1) * HPC)
        in_maps.append(
            {
                "q": np.ascontiguousarray(q[hs]),
                "k": np.ascontiguousarray(k[hs]),
                "v": np.ascontiguousarray(v[hs]),
                "mask": np.ascontiguousarray(mask),
            }
        )
    out = run_bass_kernel_spmd(nc, in_maps, core_ids=list(range(N_CORES)), trace=trace, **kw)
    return out.results, out


def kernel(query_layer, key_layer, value_layer, attention_mask):
    q = np.asarray(query_layer, dtype=np.float32)
    k = np.asarray(key_layer, dtype=np.float32)
    v = np.asarray(value_layer, dtype=np.float32)
    mask = np.asarray(attention_mask, dtype=np.float32).reshape(-1)
    B, Hn, S, Dn = q.shape
    assert (B, Hn, S, Dn) == (1, H, FULL_S, D), (B, Hn, S, Dn)

    results, _ = run_cores(q[0], k[0], v[0], mask)

    ctx = np.concatenate([r["ctx"] for r in results], axis=0).reshape(1, H, S, D)
    lse = np.concatenate([r["lse"] for r in results], axis=0).reshape(1, H, S, 1)
    return ctx.astype(np.float32), lse.astype(np.float32)
